# revision 30
# baseline (speedup 1.0000x reference)
"""Trainium2 Bass kernel for the Centroid (segment_reduce) problem.

new_centroid = 0.3 * (segment_sum(embed, y) / counts) + 0.7 * centroid
  embed [32768, 1024] f32, y [32768] int64 (0..999), centroid [1000, 1024] f32

Strategy (8 NeuronCores, 4 batch-shards x 2 class-halves grid):
  - core i = (r, h) with r = i % 4 (batch shard, 8192 rows) and
    h = i // 4 (class half, 512 classes). Each core sees embed rows
    [8192*r, 8192*(r+1)) as fp8 e4m3 laid out [128, 64, 1024] on the
    host, plus y - 512*h as f32.
  - scatter-add as a dense one-hot matmul on TensorE in fp8 DoubleRow
    mode: sums[c, d] = sum_b onehot[b, c] * embed[b, d] for the core's
    512 classes only. A trailing ones column gives per-class counts.
  - PE work (B*C*D) is grid-invariant, but the cross-core reduce
    shrinks vs 8-way batch sharding: one ReduceScatter over 4 cores of
    [512, W] in fp8 (1/4 the wire bytes of the 8-way bf16 scheme; the
    fp8 rounding of sums/counts adds ~5e-3 rel err against a 2e-2
    budget).
  - 2 big PSUM passes (512+512 cols, 4 class tiles x 1 bank each = all
    8 banks) + a tiny 2-col count pass; the tile scheduler interleaves
    them behind the embed DMA pipeline.
  - a single RS right after the matmuls beats a hidden+exposed pair:
    each RS costs ~15us flat on the collective cores, and with only
    ~28us of PE there isn't enough compute to hide one.
  - finalize: count column is fetched first (128B DMA) so the
    reciprocal overlaps the payload DMAs; then per column half
    mean = sums * (0.3/count), out = mean + 0.7*centroid.
  - host concatenates the 8 [128, 1024] shards and trims to 1000 rows.
"""

import numpy as np

import concourse.bacc as bacc
import concourse.mybir as mybir
import concourse.tile as tile
from concourse.bass_utils import run_bass_kernel_spmd

N_CORES = 8
R = 4  # batch shards
H = 2  # class halves
C = 1000  # real classes
C_PAD = 1024  # padded classes
C_LOC = C_PAD // H  # 512 classes per core
D = 1024  # embed dim
B = 32768  # total batch
B_LOC = B // R  # 8192 rows per core
P = 128
KT = B_LOC // P  # 64 k-tiles per core
KP = KT // 2  # 32 k-pairs; DoubleRow consumes [128, 2, cols] per matmul
MT = C_LOC // P  # 4 class tiles
CM = C_PAD // N_CORES  # 128 classes owned per core after ReduceScatter
NG = 8  # embed arrives in 8 group DMAs of 8 k-tiles (1 MB) each
KT_G = KT // NG
FACTOR = 0.3
W = D + 2  # sums + count col + pad -> 1026 cols (count at col 1024)
CNT = D  # count column index
# PSUM passes: two 512-col passes (one bank per class tile, 8 banks
# total); the 2-col count pass runs first, interleaved into the embed
# DMA ramp (it only needs the one-hots), so its eviction never trails
# the big matmuls
CHUNKS = [(0, 512), (512, 512)]
GROUPS = [[0, 1, 2, 3], [4, 5, 6, 7]]

_F32 = mybir.dt.float32
_FP16 = mybir.dt.float16
_FP8 = mybir.dt.float8e4

_COLL_DT = _FP8  # collective payload dtype (flip to bf16 if fp8 RS breaks)

_CACHE: dict = {}


def _build():
    nc = bacc.Bacc(
        "TRN2", target_bir_lowering=False, debug=False, num_devices=N_CORES
    )
    # embed pre-laid out on host as [128, 64, 1024]: (p, k, c) = row 128k+p
    embed8 = nc.dram_tensor(
        "embed8", [P, KT, D], _FP8, kind="ExternalInput"
    ).ap()
    yt = nc.dram_tensor("yt", [P, KT], _F32, kind="ExternalInput").ap()
    cent = nc.dram_tensor("cent", [CM, D], _F32, kind="ExternalInput").ap()
    out = nc.dram_tensor("out", [CM, D], _F32, kind="ExternalOutput").ap()

    with tile.TileContext(nc) as tc:
        with (
            tc.tile_pool(name="dram", bufs=1, space="DRAM") as dram,
            tc.tile_pool(name="const", bufs=1) as const_pool,
            tc.tile_pool(name="stage", bufs=8) as stage_pool,
            tc.tile_pool(name="psum", bufs=8, space="PSUM") as psum_pool,
            tc.tile_pool(name="fin", bufs=3) as fin_pool,
        ):
            cc_in = dram.tile([C_LOC, W], _COLL_DT, name="cc_in")
            cc_out = dram.tile([CM, W], _COLL_DT, name="cc_out")

            # all 64 k-tiles' labels in one DMA: y_all[:, k] = y[k*128:(k+1)*128]
            y_all = const_pool.tile([P, KT], _F32)
            nc.gpsimd.dma_start(out=y_all[:], in_=yt[:])
            # iota row replicated down all 128 partitions: iota[p, c] = c
            # (fp16 is exact for ids < 2048)
            iota = const_pool.tile([P, C_LOC], _FP16)
            nc.gpsimd.iota(
                iota[:],
                pattern=[[1, C_LOC]],
                base=0,
                channel_multiplier=0,
                allow_small_or_imprecise_dtypes=True,
            )

            # one big embed tile; count column + pad set once, data arrives
            # in NG group DMAs so pass-0 matmuls pipeline behind them
            emb_t = const_pool.tile([P, KT, W], _FP8, name="embt")
            nc.vector.memset(emb_t[:, :, D:W], 1.0)  # count col (+1.0 pad)
            # first two groups are small so the first matmul starts ~1.5us
            # earlier; the rest are 1MB for issue efficiency
            group_kts = [4, 4] + [8] * 7
            group_lo = [sum(group_kts[:i]) for i in range(len(group_kts))]
            for g, (lo, n_kt) in enumerate(zip(group_lo, group_kts)):
                ks = slice(lo, lo + n_kt)
                nc.sync.dma_start(out=emb_t[:, ks, 0:D], in_=embed8[:, ks, :])

            # one-hots for all 64 k-tiles, fp8 (exact): oh[p, k, c] = (y==c)
            oh_t = const_pool.tile([P, KT, C_LOC], _FP8, name="oht")
            for k in range(KT):
                nc.vector.tensor_scalar(
                    oh_t[:, k, :],
                    iota[:],
                    y_all[:, k : k + 1],
                    None,
                    mybir.AluOpType.is_equal,
                )

            # pre-scale the centroid by 0.7 while the matmuls run
            c_sb = fin_pool.tile([P, D], _F32, name="c07", tag="c07", bufs=1)
            nc.gpsimd.dma_start(out=c_sb[:], in_=cent[:])
            nc.scalar.mul(c_sb[:], c_sb[:], 1.0 - FACTOR)

            def make_mm(off, n, psums):
                def mm(j, m):
                    nc.tensor.matmul(
                        psums[m][:],
                        lhsT=oh_t[:, 2 * j : 2 * j + 2, m * P : (m + 1) * P],
                        rhs=emb_t[:, 2 * j : 2 * j + 2, off : off + n],
                        start=(j == 0),
                        stop=(j == KP - 1),
                        perf_mode=mybir.MatmulPerfMode.DoubleRow,
                    )

                return mm

            # evict a pass into the collective buffer's column range,
            # split across ACT and DVE; eviction DMAs are issued from the
            # SP/ACT queues, never from Pool (which must stay free for
            # the collective)
            def evict(p, psums):
                for m in range(MT):
                    sums_sb = stage_pool.tile(
                        [P, 512], _COLL_DT, name=f"sb{p}_{m}", tag="sums_sb"
                    )
                    if (p + m) % 2 == 0:
                        nc.vector.tensor_copy(out=sums_sb[:], in_=psums[m][:])
                        dma_eng = nc.sync
                    else:
                        nc.scalar.copy(out=sums_sb[:], in_=psums[m][:])
                        dma_eng = nc.scalar
                    dma_eng.dma_start(
                        out=cc_in[m * P : (m + 1) * P, p * 512 : p * 512 + 512],
                        in_=sums_sb[:],
                    )

            # PSUM slot plan (tag "ps", 8 bank slots): pass 0 -> slots
            # 0-3, pass 1 -> 4-7 (hoistable into pass-0's DMA gaps from
            # t=0), count tiles cycle back onto 0-3 once pass 0 evicts
            psums0 = [
                psum_pool.tile([P, 512], _F32, name=f"ps0_{m}", tag="ps")
                for m in range(MT)
            ]
            psums1 = [
                psum_pool.tile([P, 512], _F32, name=f"ps1_{m}", tag="ps")
                for m in range(MT)
            ]
            cpsums = [
                psum_pool.tile([P, 2], _F32, name=f"cps{m}", tag="ps")
                for m in range(MT)
            ]
            mm0 = make_mm(0, 512, psums0)
            mm1 = make_mm(512, 512, psums1)
            cmm = make_mm(D, 2, cpsums)

            # group-major: tracks the embed DMA pipeline (group g's
            # matmuls only need group-DMA g, not all of embed)
            for lo, n_kt in zip(group_lo, group_kts):
                js = range(lo // 2, (lo + n_kt) // 2)
                for m in range(MT):
                    for j in js:
                        mm0(j, m)

            evict(0, psums0)

            # m-major: staggered PSUM eviction so copies/DMAs overlap
            for m in range(MT):
                for j in range(KP):
                    mm1(j, m)
            evict(1, psums1)

            # count matmuls last (emitting them earlier blocks the
            # scheduler from hoisting pass-1 matmuls into pass-0's gaps)
            for j in range(KP):
                for m in range(MT):
                    cmm(j, m)
            # pack the 4 count psums into one stage tile -> single DMA
            csum_sb = stage_pool.tile([P, MT, 2], _COLL_DT, name="csum")
            for m in range(MT):
                nc.scalar.copy(out=csum_sb[:, m, :], in_=cpsums[m][:])
            nc.scalar.dma_start(
                out=cc_in[:, D : D + 2].rearrange("(m p) c -> p m c", m=MT),
                in_=csum_sb[:],
            )


            nc.gpsimd.collective_compute(
                "ReduceScatter",
                mybir.AluOpType.add,
                replica_groups=GROUPS,
                ins=[cc_in.opt()],
                outs=[cc_out.opt()],
            )

            # finalize: mean = sums * (0.3/count); out = mean + 0.7*centroid.
            # count column first (128B DMA) so the reciprocal overlaps the
            # payload halves' DMAs; halves pipeline DVE work with out DMAs.
            cnt_sb = fin_pool.tile([P, 1], _COLL_DT, name="cnt", bufs=1)
            nc.scalar.dma_start(out=cnt_sb[:], in_=cc_out[:, CNT : CNT + 1])
            recip = fin_pool.tile([P, 1], _F32, name="recip", bufs=1)
            nc.vector.reciprocal(recip[:], cnt_sb[:])
            nc.vector.tensor_scalar(
                recip[:], recip[:], FACTOR, None, mybir.AluOpType.mult
            )
            NQ = 2
            for q in range(NQ):
                d_lo, ncols = q * D // NQ, D // NQ
                red = fin_pool.tile(
                    [P, ncols], _COLL_DT, name=f"red{q}", tag="red", bufs=2
                )
                red_eng = nc.sync if q % 2 == 0 else nc.scalar
                red_eng.dma_start(out=red[:], in_=cc_out[:, d_lo : d_lo + ncols])
                t1 = fin_pool.tile([P, ncols], _F32, name=f"t1_{q}", tag="t1")
                nc.scalar.mul(t1[:], red[:], recip[:, 0:1])
                out_sb = fin_pool.tile([P, ncols], _F32, name=f"o{q}", tag="o")
                nc.vector.tensor_tensor(
                    out=out_sb[:],
                    in0=t1[:],
                    in1=c_sb[:, d_lo : d_lo + ncols],
                    op=mybir.AluOpType.add,
                )
                out_eng = nc.sync if q % 2 == 0 else nc.gpsimd
                out_eng.dma_start(out=out[:, d_lo : d_lo + ncols], in_=out_sb[:])

    nc.compile()
    return nc


def get_nc():
    if "nc" not in _CACHE:
        _CACHE["nc"] = _build()
    return _CACHE["nc"]


def make_in_maps(embed: np.ndarray, y: np.ndarray, centroid: np.ndarray):
    fp8_np = mybir.dt.np(_FP8)
    embed8 = np.ascontiguousarray(embed, dtype=np.float32).astype(fp8_np)
    # [B, D] -> per shard [128, 64, 1024]: (p, k, c) = shard row 128k+p
    embed8 = embed8.reshape(R, KT, P, D).transpose(0, 2, 1, 3)
    embed8 = np.ascontiguousarray(embed8)
    y_f = np.asarray(y).astype(np.float32)
    cent_pad = np.zeros((C_PAD, D), dtype=np.float32)
    cent_pad[:C] = np.asarray(centroid, dtype=np.float32)
    in_maps = []
    for i in range(N_CORES):
        r, h = i % R, i // R
        y_loc = y_f[r * B_LOC : (r + 1) * B_LOC] - np.float32(h * C_LOC)
        in_maps.append(
            {
                "embed8": embed8[r],
                # yt[:, k] = y_loc[k*128:(k+1)*128]
                "yt": np.ascontiguousarray(y_loc.reshape(KT, P).T),
                "cent": np.ascontiguousarray(cent_pad[i * CM : (i + 1) * CM]),
            }
        )
    return in_maps


def kernel(embed: np.ndarray, y: np.ndarray, centroid: np.ndarray) -> np.ndarray:
    nc = get_nc()
    in_maps = make_in_maps(embed, y, centroid)
    res = run_bass_kernel_spmd(nc, in_maps, core_ids=list(range(N_CORES)))
    full = np.concatenate([res.results[i]["out"] for i in range(N_CORES)], axis=0)
    return np.ascontiguousarray(full[:C]).astype(np.float32)


# revision 32
# speedup vs baseline: 1.0859x; 1.0859x over previous
"""Trainium2 Bass kernel for the Centroid (segment_reduce) problem.

new_centroid = 0.3 * (segment_sum(embed, y) / counts) + 0.7 * centroid
  embed [32768, 1024] f32, y [32768] int64 (0..999), centroid [1000, 1024] f32

Strategy (8 NeuronCores, 4 batch-shards x 2 class-halves grid):
  - core i = (r, h) with r = i % 4 (batch shard, 8192 rows) and
    h = i // 4 (class half, 512 classes). Each core sees embed rows
    [8192*r, 8192*(r+1)) as fp8 e4m3 laid out [128, 64, 1024] on the
    host, plus y - 512*h as f32.
  - scatter-add as a dense one-hot matmul on TensorE in fp8 DoubleRow
    mode: sums[c, d] = sum_b onehot[b, c] * embed[b, d] for the core's
    512 classes only. A trailing ones column gives per-class counts.
  - PE work (B*C*D) is grid-invariant, but the cross-core reduce
    shrinks vs 8-way batch sharding: one ReduceScatter over 4 cores of
    [512, W] in fp8 (1/4 the wire bytes of the 8-way bf16 scheme; the
    fp8 rounding of sums/counts adds ~5e-3 rel err against a 2e-2
    budget).
  - 2 big PSUM passes (512+512 cols, 4 class tiles x 1 bank each = all
    8 banks) + a tiny 2-col count pass; the tile scheduler interleaves
    them behind the embed DMA pipeline.
  - a single RS right after the matmuls beats a hidden+exposed pair:
    each RS costs ~15us flat on the collective cores, and with only
    ~28us of PE there isn't enough compute to hide one.
  - finalize: count column is fetched first (128B DMA) so the
    reciprocal overlaps the payload DMAs; then per column half
    mean = sums * (0.3/count), out = mean + 0.7*centroid.
  - host concatenates the 8 [128, 1024] shards and trims to 1000 rows.
"""

import numpy as np

import concourse.bacc as bacc
import concourse.mybir as mybir
import concourse.tile as tile
from concourse.bass_utils import run_bass_kernel_spmd

N_CORES = 8
R = 4  # batch shards
H = 2  # class halves
C = 1000  # real classes
C_PAD = 1024  # padded classes
C_LOC = C_PAD // H  # 512 classes per core
D = 1024  # embed dim
B = 32768  # total batch
B_LOC = B // R  # 8192 rows per core
P = 128
KT = B_LOC // P  # 64 k-tiles per core
KP = KT // 2  # 32 k-pairs; DoubleRow consumes [128, 2, cols] per matmul
MT = C_LOC // P  # 4 class tiles
CM = C_PAD // N_CORES  # 128 classes owned per core after ReduceScatter
NG = 8  # embed arrives in 8 group DMAs of 8 k-tiles (1 MB) each
KT_G = KT // NG
FACTOR = 0.3
W = 1040  # sums + count col + pad, 16B-aligned rows (count at col 1024)
CNT = D  # count column index
# PSUM passes: three passes, one bank per class tile (12 tiles cycle
# the 8 banks). The count column rides inside pass 2 as a regular
# column -- a separate 2-col count pass costs ~16us of unhidden
# LDWEIGHTS on real hardware (128 tiny matmuls at ~127ns each)
CHUNKS = [(0, 384), (384, 384), (768, 272)]
GROUPS = [[0, 1, 2, 3], [4, 5, 6, 7]]

_F32 = mybir.dt.float32
_FP16 = mybir.dt.float16
_FP8 = mybir.dt.float8e4

_COLL_DT = mybir.dt.bfloat16  # collective payload dtype

_CACHE: dict = {}


def _build():
    nc = bacc.Bacc(
        "TRN2", target_bir_lowering=False, debug=False, num_devices=N_CORES
    )
    # embed pre-laid out on host as [128, 64, 1024]: (p, k, c) = row 128k+p
    embed8 = nc.dram_tensor(
        "embed8", [P, KT, D], _FP8, kind="ExternalInput"
    ).ap()
    yt = nc.dram_tensor("yt", [P, KT], _F32, kind="ExternalInput").ap()
    cent = nc.dram_tensor("cent", [CM, D], _F32, kind="ExternalInput").ap()
    out = nc.dram_tensor("out", [CM, D], _F32, kind="ExternalOutput").ap()

    with tile.TileContext(nc) as tc:
        with (
            tc.tile_pool(name="dram", bufs=1, space="DRAM") as dram,
            tc.tile_pool(name="const", bufs=1) as const_pool,
            tc.tile_pool(name="stage", bufs=8) as stage_pool,
            tc.tile_pool(name="psum", bufs=8, space="PSUM") as psum_pool,
            tc.tile_pool(name="fin", bufs=3) as fin_pool,
        ):
            cc_in = dram.tile([C_LOC, W], _COLL_DT, name="cc_in")
            cc_out = dram.tile([CM, W], _COLL_DT, name="cc_out")

            # y DMA on the SP queue (ahead of the embed groups) so the
            # gpsimd queue can run iota immediately -- serializing these
            # two on one queue costs ~2us of ramp on real hardware
            y_all = const_pool.tile([P, KT], _F32)
            nc.sync.dma_start(out=y_all[:], in_=yt[:])
            # iota row replicated down all 128 partitions: iota[p, c] = c
            # (fp16 is exact for ids < 2048)
            iota = const_pool.tile([P, C_LOC], _FP16)
            nc.gpsimd.iota(
                iota[:],
                pattern=[[1, C_LOC]],
                base=0,
                channel_multiplier=0,
                allow_small_or_imprecise_dtypes=True,
            )

            # one big embed tile; count column + pad set once, data arrives
            # in NG group DMAs so pass-0 matmuls pipeline behind them
            emb_t = const_pool.tile([P, KT, W], _FP8, name="embt")
            nc.vector.memset(emb_t[:, :, D : D + 1], 1.0)  # count col
            nc.vector.memset(emb_t[:, :, D + 1 : W], 0.0)  # pad
            # first two groups are small so the first matmul starts ~1.5us
            # earlier; the rest are 1MB for issue efficiency
            group_kts = [4, 4] + [8] * 7
            group_lo = [sum(group_kts[:i]) for i in range(len(group_kts))]
            for g, (lo, n_kt) in enumerate(zip(group_lo, group_kts)):
                ks = slice(lo, lo + n_kt)
                nc.sync.dma_start(out=emb_t[:, ks, 0:D], in_=embed8[:, ks, :])

            # one-hots for all 64 k-tiles, fp8 (exact): oh[p, k, c] = (y==c)
            oh_t = const_pool.tile([P, KT, C_LOC], _FP8, name="oht")
            for k in range(KT):
                nc.vector.tensor_scalar(
                    oh_t[:, k, :],
                    iota[:],
                    y_all[:, k : k + 1],
                    None,
                    mybir.AluOpType.is_equal,
                )

            # pre-scale the centroid by 0.7 while the matmuls run
            c_sb = fin_pool.tile([P, D], _F32, name="c07", tag="c07", bufs=1)
            nc.gpsimd.dma_start(out=c_sb[:], in_=cent[:])
            nc.scalar.mul(c_sb[:], c_sb[:], 1.0 - FACTOR)

            def make_mm(off, n, psums):
                def mm(j, m):
                    nc.tensor.matmul(
                        psums[m][:],
                        lhsT=oh_t[:, 2 * j : 2 * j + 2, m * P : (m + 1) * P],
                        rhs=emb_t[:, 2 * j : 2 * j + 2, off : off + n],
                        start=(j == 0),
                        stop=(j == KP - 1),
                        perf_mode=mybir.MatmulPerfMode.DoubleRow,
                    )

                return mm

            # evict a pass into the collective buffer's column range,
            # split across ACT and DVE; eviction DMAs are issued from the
            # SP/ACT queues, never from Pool (which must stay free for
            # the collective)
            def evict(p, off, n, psums):
                for m in range(MT):
                    sums_sb = stage_pool.tile(
                        [P, n], _COLL_DT, name=f"sb{p}_{m}", tag="sums_sb"
                    )
                    if (p + m) % 2 == 0:
                        nc.vector.tensor_copy(out=sums_sb[:], in_=psums[m][:])
                        dma_eng = nc.sync
                    else:
                        nc.scalar.copy(out=sums_sb[:], in_=psums[m][:])
                        dma_eng = nc.scalar
                    dma_eng.dma_start(
                        out=cc_in[m * P : (m + 1) * P, off : off + n],
                        in_=sums_sb[:],
                    )

            # PSUM slot plan (tag "ps", 8 bank slots): pass 0 -> slots
            # 0-3, pass 1 -> 4-7 (hoistable into pass-0's DMA gaps from
            # t=0), pass 2 cycles back onto 0-3 once pass 0 evicts
            all_psums = []
            for p, (off, n) in enumerate(CHUNKS):
                all_psums.append(
                    [
                        psum_pool.tile([P, n], _F32, name=f"ps{p}_{m}", tag="ps")
                        for m in range(MT)
                    ]
                )
            mms = [
                make_mm(off, n, all_psums[p])
                for p, (off, n) in enumerate(CHUNKS)
            ]

            # pass 0 group-major: tracks the embed DMA pipeline (group
            # g's matmuls only need group-DMA g, not all of embed)
            for lo, n_kt in zip(group_lo, group_kts):
                js = range(lo // 2, (lo + n_kt) // 2)
                for m in range(MT):
                    for j in js:
                        mms[0](j, m)
            evict(0, CHUNKS[0][0], CHUNKS[0][1], all_psums[0])

            # passes 1+2 m-major: staggered PSUM eviction
            for p in (1, 2):
                for m in range(MT):
                    for j in range(KP):
                        mms[p](j, m)
                evict(p, CHUNKS[p][0], CHUNKS[p][1], all_psums[p])


            nc.gpsimd.collective_compute(
                "ReduceScatter",
                mybir.AluOpType.add,
                replica_groups=GROUPS,
                ins=[cc_in.opt()],
                outs=[cc_out.opt()],
            )

            # finalize: mean = sums * (0.3/count); out = mean + 0.7*centroid.
            # count column first (128B DMA) so the reciprocal overlaps the
            # payload halves' DMAs; halves pipeline DVE work with out DMAs.
            cnt_sb = fin_pool.tile([P, 1], _COLL_DT, name="cnt", bufs=1)
            nc.scalar.dma_start(out=cnt_sb[:], in_=cc_out[:, CNT : CNT + 1])
            recip = fin_pool.tile([P, 1], _F32, name="recip", bufs=1)
            nc.vector.reciprocal(recip[:], cnt_sb[:])
            nc.vector.tensor_scalar(
                recip[:], recip[:], FACTOR, None, mybir.AluOpType.mult
            )
            NQ = 2
            for q in range(NQ):
                d_lo, ncols = q * D // NQ, D // NQ
                red = fin_pool.tile(
                    [P, ncols], _COLL_DT, name=f"red{q}", tag="red", bufs=2
                )
                red_eng = nc.sync if q % 2 == 0 else nc.scalar
                red_eng.dma_start(out=red[:], in_=cc_out[:, d_lo : d_lo + ncols])
                t1 = fin_pool.tile([P, ncols], _F32, name=f"t1_{q}", tag="t1")
                nc.scalar.mul(t1[:], red[:], recip[:, 0:1])
                out_sb = fin_pool.tile([P, ncols], _F32, name=f"o{q}", tag="o")
                nc.vector.tensor_tensor(
                    out=out_sb[:],
                    in0=t1[:],
                    in1=c_sb[:, d_lo : d_lo + ncols],
                    op=mybir.AluOpType.add,
                )
                out_eng = nc.sync if q % 2 == 0 else nc.gpsimd
                out_eng.dma_start(out=out[:, d_lo : d_lo + ncols], in_=out_sb[:])

    nc.compile()
    return nc


def get_nc():
    if "nc" not in _CACHE:
        _CACHE["nc"] = _build()
    return _CACHE["nc"]


def make_in_maps(embed: np.ndarray, y: np.ndarray, centroid: np.ndarray):
    fp8_np = mybir.dt.np(_FP8)
    embed8 = np.ascontiguousarray(embed, dtype=np.float32).astype(fp8_np)
    # [B, D] -> per shard [128, 64, 1024]: (p, k, c) = shard row 128k+p
    embed8 = embed8.reshape(R, KT, P, D).transpose(0, 2, 1, 3)
    embed8 = np.ascontiguousarray(embed8)
    y_f = np.asarray(y).astype(np.float32)
    cent_pad = np.zeros((C_PAD, D), dtype=np.float32)
    cent_pad[:C] = np.asarray(centroid, dtype=np.float32)
    in_maps = []
    for i in range(N_CORES):
        r, h = i % R, i // R
        y_loc = y_f[r * B_LOC : (r + 1) * B_LOC] - np.float32(h * C_LOC)
        in_maps.append(
            {
                "embed8": embed8[r],
                # yt[:, k] = y_loc[k*128:(k+1)*128]
                "yt": np.ascontiguousarray(y_loc.reshape(KT, P).T),
                "cent": np.ascontiguousarray(cent_pad[i * CM : (i + 1) * CM]),
            }
        )
    return in_maps


def kernel(embed: np.ndarray, y: np.ndarray, centroid: np.ndarray) -> np.ndarray:
    nc = get_nc()
    in_maps = make_in_maps(embed, y, centroid)
    res = run_bass_kernel_spmd(nc, in_maps, core_ids=list(range(N_CORES)))
    full = np.concatenate([res.results[i]["out"] for i in range(N_CORES)], axis=0)
    return np.ascontiguousarray(full[:C]).astype(np.float32)


# revision 33
# speedup vs baseline: 1.1251x; 1.0361x over previous
"""Trainium2 Bass kernel for the Centroid (segment_reduce) problem.

new_centroid = 0.3 * (segment_sum(embed, y) / counts) + 0.7 * centroid
  embed [32768, 1024] f32, y [32768] int64 (0..999), centroid [1000, 1024] f32

Strategy (8 NeuronCores, 4 batch-shards x 2 class-halves grid):
  - core i = (r, h) with r = i % 4 (batch shard, 8192 rows) and
    h = i // 4 (class half, 512 classes). Each core sees embed rows
    [8192*r, 8192*(r+1)) as fp8 e4m3 laid out [128, 64, 1024] on the
    host, plus y - 512*h as f32.
  - scatter-add as a dense one-hot matmul on TensorE in fp8 DoubleRow
    mode: sums[c, d] = sum_b onehot[b, c] * embed[b, d] for the core's
    512 classes only. A trailing ones column gives per-class counts.
  - PE work (B*C*D) is grid-invariant, but the cross-core reduce
    shrinks vs 8-way batch sharding: one ReduceScatter over 4 cores of
    [512, W] in fp8 (1/4 the wire bytes of the 8-way bf16 scheme; the
    fp8 rounding of sums/counts adds ~5e-3 rel err against a 2e-2
    budget).
  - 2 big PSUM passes (512+512 cols, 4 class tiles x 1 bank each = all
    8 banks) + a tiny 2-col count pass; the tile scheduler interleaves
    them behind the embed DMA pipeline.
  - a single RS right after the matmuls beats a hidden+exposed pair:
    each RS costs ~15us flat on the collective cores, and with only
    ~28us of PE there isn't enough compute to hide one.
  - finalize: count column is fetched first (128B DMA) so the
    reciprocal overlaps the payload DMAs; then per column half
    mean = sums * (0.3/count), out = mean + 0.7*centroid.
  - host concatenates the 8 [128, 1024] shards and trims to 1000 rows.
"""

import numpy as np

import concourse.bacc as bacc
import concourse.mybir as mybir
import concourse.tile as tile
from concourse.bass_utils import run_bass_kernel_spmd

N_CORES = 8
R = 4  # batch shards
H = 2  # class halves
C = 1000  # real classes
C_PAD = 1024  # padded classes
C_LOC = C_PAD // H  # 512 classes per core
D = 1024  # embed dim
B = 32768  # total batch
B_LOC = B // R  # 8192 rows per core
P = 128
KT = B_LOC // P  # 64 k-tiles per core
KP = KT // 2  # 32 k-pairs; DoubleRow consumes [128, 2, cols] per matmul
MT = C_LOC // P  # 4 class tiles
CM = C_PAD // N_CORES  # 128 classes owned per core after ReduceScatter
NG = 8  # embed arrives in 8 group DMAs of 8 k-tiles (1 MB) each
KT_G = KT // NG
FACTOR = 0.3
W = 1040  # sums + count col + pad, 16B-aligned rows (count at col 1024)
CNT = D  # count column index
# PSUM passes: three passes, one bank per class tile (12 tiles cycle
# the 8 banks). The count column rides inside pass 2 as a regular
# column -- a separate 2-col count pass costs ~16us of unhidden
# LDWEIGHTS on real hardware (128 tiny matmuls at ~127ns each)
CHUNKS = [(0, 384), (384, 384), (768, 272)]
GROUPS = [[0, 1, 2, 3], [4, 5, 6, 7]]

_F32 = mybir.dt.float32
_FP16 = mybir.dt.float16
_FP8 = mybir.dt.float8e4

_COLL_DT = mybir.dt.bfloat16  # collective payload dtype

_CACHE: dict = {}


def _build():
    nc = bacc.Bacc(
        "TRN2", target_bir_lowering=False, debug=False, num_devices=N_CORES
    )
    # embed pre-laid out on host as [128, 64, 1024]: (p, k, c) = row 128k+p
    embed8 = nc.dram_tensor(
        "embed8", [P, KT, D], _FP8, kind="ExternalInput"
    ).ap()
    yt = nc.dram_tensor("yt", [P, KT], _F32, kind="ExternalInput").ap()
    cent = nc.dram_tensor("cent", [CM, D], _F32, kind="ExternalInput").ap()
    out = nc.dram_tensor("out", [CM, D], _F32, kind="ExternalOutput").ap()

    with tile.TileContext(nc) as tc:
        with (
            tc.tile_pool(name="dram", bufs=1, space="DRAM") as dram,
            tc.tile_pool(name="const", bufs=1) as const_pool,
            tc.tile_pool(name="stage", bufs=8) as stage_pool,
            tc.tile_pool(name="psum", bufs=8, space="PSUM") as psum_pool,
            tc.tile_pool(name="fin", bufs=3) as fin_pool,
        ):
            # packed collective layout: 4 classes per row (class c at row
            # c//4, column block (c%4)*W) -- the RS scatter unit over a
            # 4-rank group is rows/4, so core k's 32 rows hold exactly its
            # 128 classes, and the NRT launch cost (descriptor-bound, ~us
            # per 128 rows) drops 4x vs a [512, W] payload
            cc_in = dram.tile([C_LOC // MT, MT * W], _COLL_DT, name="cc_in")
            cc_out = dram.tile([CM // MT, MT * W], _COLL_DT, name="cc_out")

            # y DMA on the SP queue (ahead of the embed groups) so the
            # gpsimd queue can run iota immediately -- serializing these
            # two on one queue costs ~2us of ramp on real hardware
            y_all = const_pool.tile([P, KT], _F32)
            nc.sync.dma_start(out=y_all[:], in_=yt[:])
            # iota row replicated down all 128 partitions: iota[p, c] = c
            # (fp16 is exact for ids < 2048)
            iota = const_pool.tile([P, C_LOC], _FP16)
            nc.gpsimd.iota(
                iota[:],
                pattern=[[1, C_LOC]],
                base=0,
                channel_multiplier=0,
                allow_small_or_imprecise_dtypes=True,
            )

            # one big embed tile; count column + pad set once, data arrives
            # in NG group DMAs so pass-0 matmuls pipeline behind them
            emb_t = const_pool.tile([P, KT, W], _FP8, name="embt")
            nc.vector.memset(emb_t[:, :, D : D + 1], 1.0)  # count col
            nc.vector.memset(emb_t[:, :, D + 1 : W], 0.0)  # pad
            # first two groups are small so the first matmul starts ~1.5us
            # earlier; the rest are 1MB for issue efficiency
            group_kts = [4, 4] + [8] * 7
            group_lo = [sum(group_kts[:i]) for i in range(len(group_kts))]
            for g, (lo, n_kt) in enumerate(zip(group_lo, group_kts)):
                ks = slice(lo, lo + n_kt)
                nc.sync.dma_start(out=emb_t[:, ks, 0:D], in_=embed8[:, ks, :])

            # one-hots for all 64 k-tiles, fp8 (exact): oh[p, k, c] = (y==c)
            oh_t = const_pool.tile([P, KT, C_LOC], _FP8, name="oht")
            for k in range(KT):
                nc.vector.tensor_scalar(
                    oh_t[:, k, :],
                    iota[:],
                    y_all[:, k : k + 1],
                    None,
                    mybir.AluOpType.is_equal,
                )

            # pre-scale the centroid by 0.7 while the matmuls run
            c_sb = fin_pool.tile([P, D], _F32, name="c07", tag="c07", bufs=1)
            nc.gpsimd.dma_start(out=c_sb[:], in_=cent[:])
            nc.scalar.mul(c_sb[:], c_sb[:], 1.0 - FACTOR)

            def make_mm(off, n, psums):
                def mm(j, m):
                    nc.tensor.matmul(
                        psums[m][:],
                        lhsT=oh_t[:, 2 * j : 2 * j + 2, m * P : (m + 1) * P],
                        rhs=emb_t[:, 2 * j : 2 * j + 2, off : off + n],
                        start=(j == 0),
                        stop=(j == KP - 1),
                        perf_mode=mybir.MatmulPerfMode.DoubleRow,
                    )

                return mm

            # evict a pass into the collective buffer's column range,
            # split across ACT and DVE; eviction DMAs are issued from the
            # SP/ACT queues, never from Pool (which must stay free for
            # the collective)
            def evict(p, off, n, psums):
                for m in range(MT):
                    sums_sb = stage_pool.tile(
                        [P, n], _COLL_DT, name=f"sb{p}_{m}", tag="sums_sb"
                    )
                    if (p + m) % 2 == 0:
                        nc.vector.tensor_copy(out=sums_sb[:], in_=psums[m][:])
                        dma_eng = nc.sync
                    else:
                        nc.scalar.copy(out=sums_sb[:], in_=psums[m][:])
                        dma_eng = nc.scalar
                    rows = P // MT
                    dst = cc_in[m * rows : (m + 1) * rows, :].rearrange(
                        "a (b c) -> a b c", b=MT
                    )[:, :, off : off + n]
                    dma_eng.dma_start(out=dst, in_=sums_sb[:])

            # PSUM slot plan (tag "ps", 8 bank slots): pass 0 -> slots
            # 0-3, pass 1 -> 4-7 (hoistable into pass-0's DMA gaps from
            # t=0), pass 2 cycles back onto 0-3 once pass 0 evicts
            all_psums = []
            for p, (off, n) in enumerate(CHUNKS):
                all_psums.append(
                    [
                        psum_pool.tile([P, n], _F32, name=f"ps{p}_{m}", tag="ps")
                        for m in range(MT)
                    ]
                )
            mms = [
                make_mm(off, n, all_psums[p])
                for p, (off, n) in enumerate(CHUNKS)
            ]

            # pass 0 group-major: tracks the embed DMA pipeline (group
            # g's matmuls only need group-DMA g, not all of embed)
            for lo, n_kt in zip(group_lo, group_kts):
                js = range(lo // 2, (lo + n_kt) // 2)
                for m in range(MT):
                    for j in js:
                        mms[0](j, m)
            evict(0, CHUNKS[0][0], CHUNKS[0][1], all_psums[0])

            # passes 1+2 m-major: staggered PSUM eviction
            for p in (1, 2):
                for m in range(MT):
                    for j in range(KP):
                        mms[p](j, m)
                evict(p, CHUNKS[p][0], CHUNKS[p][1], all_psums[p])


            nc.gpsimd.collective_compute(
                "ReduceScatter",
                mybir.AluOpType.add,
                replica_groups=GROUPS,
                ins=[cc_in.opt()],
                outs=[cc_out.opt()],
            )

            # finalize: mean = sums * (0.3/count); out = mean + 0.7*centroid.
            # count column first (128B DMA) so the reciprocal overlaps the
            # payload halves' DMAs; halves pipeline DVE work with out DMAs.
            def red_src(d_lo, ncols):
                # partitions c = 4a + b map to row a, col block b
                return cc_out[:].rearrange("a (b c) -> a b c", b=MT)[
                    :, :, d_lo : d_lo + ncols
                ]

            cnt_sb = fin_pool.tile([P, 1], _COLL_DT, name="cnt", bufs=1)
            nc.scalar.dma_start(out=cnt_sb[:], in_=red_src(CNT, 1))
            recip = fin_pool.tile([P, 1], _F32, name="recip", bufs=1)
            nc.vector.reciprocal(recip[:], cnt_sb[:])
            nc.vector.tensor_scalar(
                recip[:], recip[:], FACTOR, None, mybir.AluOpType.mult
            )
            NQ = 2
            for q in range(NQ):
                d_lo, ncols = q * D // NQ, D // NQ
                red = fin_pool.tile(
                    [P, ncols], _COLL_DT, name=f"red{q}", tag="red", bufs=2
                )
                red_eng = nc.sync if q % 2 == 0 else nc.scalar
                red_eng.dma_start(out=red[:], in_=red_src(d_lo, ncols))
                t1 = fin_pool.tile([P, ncols], _F32, name=f"t1_{q}", tag="t1")
                nc.scalar.mul(t1[:], red[:], recip[:, 0:1])
                out_sb = fin_pool.tile([P, ncols], _F32, name=f"o{q}", tag="o")
                nc.vector.tensor_tensor(
                    out=out_sb[:],
                    in0=t1[:],
                    in1=c_sb[:, d_lo : d_lo + ncols],
                    op=mybir.AluOpType.add,
                )
                out_eng = nc.sync if q % 2 == 0 else nc.gpsimd
                out_eng.dma_start(out=out[:, d_lo : d_lo + ncols], in_=out_sb[:])

    nc.compile()
    return nc


def get_nc():
    if "nc" not in _CACHE:
        _CACHE["nc"] = _build()
    return _CACHE["nc"]


def make_in_maps(embed: np.ndarray, y: np.ndarray, centroid: np.ndarray):
    fp8_np = mybir.dt.np(_FP8)
    embed8 = np.ascontiguousarray(embed, dtype=np.float32).astype(fp8_np)
    # [B, D] -> per shard [128, 64, 1024]: (p, k, c) = shard row 128k+p
    embed8 = embed8.reshape(R, KT, P, D).transpose(0, 2, 1, 3)
    embed8 = np.ascontiguousarray(embed8)
    y_f = np.asarray(y).astype(np.float32)
    cent_pad = np.zeros((C_PAD, D), dtype=np.float32)
    cent_pad[:C] = np.asarray(centroid, dtype=np.float32)
    in_maps = []
    for i in range(N_CORES):
        r, h = i % R, i // R
        y_loc = y_f[r * B_LOC : (r + 1) * B_LOC] - np.float32(h * C_LOC)
        in_maps.append(
            {
                "embed8": embed8[r],
                # yt[:, k] = y_loc[k*128:(k+1)*128]
                "yt": np.ascontiguousarray(y_loc.reshape(KT, P).T),
                "cent": np.ascontiguousarray(cent_pad[i * CM : (i + 1) * CM]),
            }
        )
    return in_maps


def kernel(embed: np.ndarray, y: np.ndarray, centroid: np.ndarray) -> np.ndarray:
    nc = get_nc()
    in_maps = make_in_maps(embed, y, centroid)
    res = run_bass_kernel_spmd(nc, in_maps, core_ids=list(range(N_CORES)))
    full = np.concatenate([res.results[i]["out"] for i in range(N_CORES)], axis=0)
    return np.ascontiguousarray(full[:C]).astype(np.float32)
